# revision 7
# baseline (speedup 1.0000x reference)
"""Trainium2 Bass kernel for nn_Attention (llama-style attention layer, fp32).

Full inputs in, full output out. Internally: tensor-parallel over heads x
data-parallel over batch on 8 NeuronCores (2 batch groups x 4 head groups,
8 heads per core), reduce-scatter after the output projection.

All matmuls run as float32r (TF32-like, ~fp22 operand precision) at full PE
rate with fp32 PSUM accumulation.
"""
import os
import sys

sys.path.insert(0, "/opt/trn_rl_repo")

import numpy as np

import concourse.bass as bass
import concourse.mybir as mybir
import concourse.tile as tile
from concourse import bacc
from concourse.bass import ts
from concourse.bass_utils import run_bass_kernel_spmd

DIM = 4096
N_HEADS = 32
HEAD_DIM = 128
B, S = 2, 2048
N_CORES = 8
N_GROUPS = 4                  # head groups (tensor parallel)
HPC = N_HEADS // N_GROUPS     # heads per core = 8
FPC = HPC * HEAD_DIM          # features per core = 1024
P = 128
KO = DIM // P                 # 32 k-tiles over the model dim
NT = S // 512                 # 4 token stripes of 512
TT = S // P                   # 16 token tiles of 128
RS_CHUNKS = 8                 # reduce-scatter chunks (256 tokens each)
CHUNK_TOK = S // RS_CHUNKS    # 256
SCALE = 1.0 / float(np.sqrt(HEAD_DIM))

f32 = mybir.dt.float32
f32r = mybir.dt.float32r
EXP = mybir.ActivationFunctionType.Exp
COPY = mybir.ActivationFunctionType.Copy
MULT = mybir.AluOpType.mult
ADD = mybir.AluOpType.add

_CACHE = {}


def _build():
    nc = bacc.Bacc(
        "TRN2", target_bir_lowering=False, debug=False, num_devices=N_CORES
    )

    xT = nc.dram_tensor("xT", [DIM, S], f32r, kind="ExternalInput")
    wqT = nc.dram_tensor("wqT", [DIM, FPC], f32r, kind="ExternalInput")
    wkT = nc.dram_tensor("wkT", [DIM, FPC], f32r, kind="ExternalInput")
    wvT = nc.dram_tensor("wvT", [DIM, FPC], f32r, kind="ExternalInput")
    woT = nc.dram_tensor("woT", [FPC, DIM], f32r, kind="ExternalInput")
    cb_d = nc.dram_tensor("cb", [P, S], f32, kind="ExternalInput")
    ss_d = nc.dram_tensor("ss", [P, S], f32, kind="ExternalInput")
    perm_d = nc.dram_tensor("perm", [P, P], f32r, kind="ExternalInput")
    ones_d = nc.dram_tensor("ones", [P, P], f32r, kind="ExternalInput")
    out_e = nc.dram_tensor("out", [S // N_GROUPS, DIM], f32, kind="ExternalOutput")

    xT3 = xT.ap().rearrange("(ko p) t -> p ko t", p=P)       # [128, 32, 2048]
    wq3 = wqT.ap().rearrange("(ko p) m -> p ko m", p=P)      # [128, 32, 1024]
    wk3 = wkT.ap().rearrange("(ko p) m -> p ko m", p=P)
    wv3 = wvT.ap().rearrange("(ko p) m -> p ko m", p=P)
    wo3 = woT.ap().rearrange("(ho p) d -> p ho d", p=P)      # [128, 8, 4096]

    with tile.TileContext(nc) as tc:
        with tc.tile_pool(name="dram", bufs=1, space="DRAM") as drp, \
             tc.tile_pool(name="const", bufs=1) as constp:
            q_d = drp.tile([FPC, S], f32r, tag="q_d", name="q_d")
            k_d = drp.tile([FPC, S], f32r, tag="k_d", name="k_d")
            v_d = drp.tile([S, FPC], f32r, tag="v_d", name="v_d")
            attn_d = drp.tile([FPC, S], f32r, tag="attn_d", name="attn_d")
            pc = [
                drp.tile([CHUNK_TOK, DIM], f32, tag=f"pc{ci}", name=f"pc{ci}")
                for ci in range(RS_CHUNKS)
            ]
            rs_out = [
                drp.tile([CHUNK_TOK // N_GROUPS, DIM], f32, tag=f"rs{ci}",
                         name=f"rs{ci}")
                for ci in range(RS_CHUNKS)
            ]

            q3 = q_d[:].rearrange("(mo p) t -> p mo t", p=P)     # [128, 8, 2048]
            k3 = k_d[:].rearrange("(mo p) t -> p mo t", p=P)
            v3 = v_d[:].rearrange("(to p) f -> p to f", p=P)     # [128, 16, 1024]
            a3 = attn_d[:].rearrange("(ho p) t -> p ho t", p=P)  # [128, 8, 2048]

            perm_sb = constp.tile([P, P], f32r, tag="perm", name="perm_sb")
            nc.sync.dma_start(perm_sb[:], perm_d.ap())
            ones_sb = constp.tile([P, P], f32r, tag="ones", name="ones_sb")
            nc.sync.dma_start(ones_sb[:], ones_d.ap())

            # ---------------- Phase 1: Q and K projections + RoPE ----------
            with tc.tile_pool(name="p1_rope", bufs=1) as ropep, \
                 tc.tile_pool(name="p1_x", bufs=12) as xp, \
                 tc.tile_pool(name="p1_w", bufs=10) as wp, \
                 tc.tile_pool(name="p1_t", bufs=4) as tp, \
                 tc.tile_pool(name="p1_ps", bufs=3, space="PSUM") as pp, \
                 tc.tile_pool(name="p1_ps2", bufs=2, space="PSUM") as pp2:
                cb_sb = ropep.tile([P, S], f32, tag="cb", name="cb_sb")
                ss_sb = ropep.tile([P, S], f32, tag="ss", name="ss_sb")
                nc.sync.dma_start(cb_sb[:], cb_d.ap())
                nc.sync.dma_start(ss_sb[:], ss_d.ap())

                for w3, dst3 in ((wq3, q3), (wk3, k3)):
                    for n in range(NT):  # 4 token stripes of 512
                        xs = [
                            xp.tile([P, 4, 512], f32r, tag="xsl", name="xs")
                            for _ in range(8)
                        ]
                        for kg in range(8):
                            nc.sync.dma_start(
                                xs[kg][:], xT3[:, ts(kg, 4), ts(n, 512)]
                            )
                        for m in range(HPC):  # 8 feature tiles (heads)
                            psum = pp.tile([P, 512], f32, tag="proj", name="psum")
                            for kg in range(8):
                                wt = wp.tile([P, 4, P], f32r, tag="wt", name="wt")
                                nc.sync.dma_start(
                                    wt[:], w3[:, ts(kg, 4), ts(m, P)]
                                )
                                for kk in range(4):
                                    k = 4 * kg + kk
                                    nc.tensor.matmul(
                                        psum[:],
                                        wt[:, kk],
                                        xs[kg][:, kk],
                                        start=(k == 0),
                                        stop=(k == KO - 1),
                                    )
                            # RoPE: out = raw * cos + swap(raw) * sin_signed
                            raw = tp.tile([P, 512], f32r, tag="raw", name="raw")
                            nc.scalar.activation(raw[:], psum[:], COPY)
                            ps_sw = pp2.tile([P, 512], f32, tag="permps",
                                             name="ps_sw")
                            nc.tensor.matmul(
                                ps_sw[:], perm_sb[:], raw[:],
                                start=True, stop=True,
                            )
                            qf = tp.tile([P, 512], f32r, tag="qf", name="qf")
                            nc.vector.tensor_tensor(
                                qf[:], raw[:], cb_sb[:, ts(n, 512)], MULT
                            )
                            tmp = tp.tile([P, 512], f32, tag="tmp", name="tmp")
                            nc.vector.tensor_tensor(
                                tmp[:], ps_sw[:], ss_sb[:, ts(n, 512)], MULT
                            )
                            nc.vector.tensor_tensor(qf[:], qf[:], tmp[:], ADD)
                            nc.sync.dma_start(dst3[:, m, ts(n, 512)], qf[:])

            # ---------------- Phase 2: V projection ------------------------
            with tc.tile_pool(name="p2_w", bufs=1) as wvp, \
                 tc.tile_pool(name="p2_x", bufs=16) as xcp, \
                 tc.tile_pool(name="p2_s", bufs=4) as vsp, \
                 tc.tile_pool(name="p2_ps", bufs=4, space="PSUM") as vpp:
                wv_res = [
                    wvp.tile([P, 4, FPC], f32r, tag=f"wv{kg}", name=f"wv_res{kg}")
                    for kg in range(8)
                ]
                for kg in range(8):
                    nc.sync.dma_start(wv_res[kg][:], wv3[:, ts(kg, 4), :])
                for t in range(TT):  # 16 token tiles of 128
                    xc = [
                        xcp.tile([P, 4, P], f32r, tag="xc", name="xc")
                        for _ in range(8)
                    ]
                    for kg in range(8):
                        nc.sync.dma_start(xc[kg][:], xT3[:, ts(kg, 4), ts(t, P)])
                    ps = [
                        vpp.tile([P, 512], f32, tag="vps", name="vps")
                        for _ in range(2)
                    ]
                    for kg in range(8):
                        for kk in range(4):
                            k = 4 * kg + kk
                            for fc in range(2):
                                nc.tensor.matmul(
                                    ps[fc][:],
                                    xc[kg][:, kk],
                                    wv_res[kg][:, kk, ts(fc, 512)],
                                    start=(k == 0),
                                    stop=(k == KO - 1),
                                )
                    for fc in range(2):
                        vs = vsp.tile([P, 512], f32r, tag="vsb", name="vs")
                        nc.scalar.activation(vs[:], ps[fc][:], COPY)
                        nc.sync.dma_start(v3[:, t, ts(fc, 512)], vs[:])

            # ---------------- Phase 3: attention per head -------------------
            with tc.tile_pool(name="p3_kqv", bufs=2) as hp, \
                 tc.tile_pool(name="p3_exp", bufs=20) as ep, \
                 tc.tile_pool(name="p3_o", bufs=4) as aop, \
                 tc.tile_pool(name="p3_ps_s", bufs=3, space="PSUM") as sps, \
                 tc.tile_pool(name="p3_ps_o", bufs=2, space="PSUM") as ops, \
                 tc.tile_pool(name="p3_ps_d", bufs=2, space="PSUM") as dps:
                for h in range(HPC):
                    kh = hp.tile([P, S], f32r, tag="kh", name="kh")
                    nc.sync.dma_start(kh[:], k3[:, h])
                    qh = hp.tile([P, S], f32r, tag="qh", name="qh")
                    nc.sync.dma_start(qh[:], q3[:, h])
                    vh = hp.tile([P, TT, P], f32r, tag="vh", name="vh")
                    nc.sync.dma_start(vh[:], v3[:, :, ts(h, P)])
                    for qt in range(NT):  # 4 qtok stripes of 512
                        ets = []
                        for kt in range(TT):  # 16 ktok tiles of 128
                            ps_s = sps.tile([P, 512], f32, tag="s", name="ps_s")
                            nc.tensor.matmul(
                                ps_s[:],
                                kh[:, ts(kt, P)],
                                qh[:, ts(qt, 512)],
                                start=True, stop=True,
                            )
                            et = ep.tile([P, 512], f32r, tag="e", name="et")
                            nc.scalar.activation(et[:], ps_s[:], EXP, scale=SCALE)
                            ets.append(et)
                        ps_o = ops.tile([P, 512], f32, tag="o", name="ps_o")
                        for kt in range(TT):
                            nc.tensor.matmul(
                                ps_o[:], vh[:, kt], ets[kt][:],
                                start=(kt == 0), stop=(kt == TT - 1),
                            )
                        ps_d = dps.tile([P, 512], f32, tag="d", name="ps_d")
                        for kt in range(TT):
                            nc.tensor.matmul(
                                ps_d[:], ones_sb[:], ets[kt][:],
                                start=(kt == 0), stop=(kt == TT - 1),
                            )
                        rec = aop.tile([P, 512], f32, tag="rec", name="rec")
                        nc.vector.reciprocal_approx_fast(rec[:], ps_d[:])
                        ao = aop.tile([P, 512], f32r, tag="ao", name="ao")
                        nc.vector.tensor_tensor(ao[:], ps_o[:], rec[:], MULT)
                        nc.sync.dma_start(a3[:, h, ts(qt, 512)], ao[:])

            # ---------------- Phase 4: WO + chunked ReduceScatter -----------
            with tc.tile_pool(name="p4_w", bufs=1) as wop, \
                 tc.tile_pool(name="p4_a", bufs=3) as ap_, \
                 tc.tile_pool(name="p4_s", bufs=10) as osp, \
                 tc.tile_pool(name="p4_ps", bufs=8, space="PSUM") as opp:
                wo_res = [
                    wop.tile([P, DIM], f32r, tag=f"wo{h}", name=f"wo_res{h}")
                    for h in range(HPC)
                ]
                for h in range(HPC):
                    nc.sync.dma_start(wo_res[h][:], wo3[:, h])
                for t in range(TT):  # 16 token tiles of 128
                    at = ap_.tile([P, HPC, P], f32r, tag="at", name="at")
                    nc.sync.dma_start(at[:], a3[:, :, ts(t, P)])
                    ps = [
                        opp.tile([P, 512], f32, tag="ops", name="ops")
                        for _ in range(8)
                    ]
                    for h in range(HPC):
                        for n in range(8):
                            nc.tensor.matmul(
                                ps[n][:],
                                at[:, h],
                                wo_res[h][:, ts(n, 512)],
                                start=(h == 0),
                                stop=(h == HPC - 1),
                            )
                    ci, sub = t // 2, t % 2
                    pc3 = pc[ci][:].rearrange("(to p) d -> p to d", p=P)
                    for n in range(8):
                        ob = osp.tile([P, 512], f32, tag="ob", name="ob")
                        nc.scalar.activation(ob[:], ps[n][:], COPY)
                        nc.sync.dma_start(pc3[:, sub, ts(n, 512)], ob[:])
                    if sub == 1:
                        nc.gpsimd.collective_compute(
                            "ReduceScatter",
                            ADD,
                            replica_groups=[[0, 1, 2, 3], [4, 5, 6, 7]],
                            ins=[pc[ci][:]],
                            outs=[rs_out[ci][:]],
                        )
                        nc.sync.dma_start(
                            out_e.ap()[ts(ci, CHUNK_TOK // N_GROUPS), :],
                            rs_out[ci][:],
                        )

    nc.compile()
    return nc


def _prep_inputs(x, freqs_cos, freqs_sin, wq, wk, wv, wo):
    x = np.asarray(x, dtype=np.float32)
    fc = np.asarray(freqs_cos, dtype=np.float32)
    fs = np.asarray(freqs_sin, dtype=np.float32)
    wq = np.asarray(wq, dtype=np.float32)
    wk = np.asarray(wk, dtype=np.float32)
    wv = np.asarray(wv, dtype=np.float32)
    wo = np.asarray(wo, dtype=np.float32)

    cb = np.repeat(fc.T, 2, axis=0)          # [128, S]: cos[t, p//2]
    ss = np.repeat(fs.T, 2, axis=0)          # [128, S]
    ss[0::2, :] *= -1.0                      # even rows: -sin, odd rows: +sin
    cb = np.ascontiguousarray(cb, dtype=np.float32)
    ss = np.ascontiguousarray(ss, dtype=np.float32)

    idx = np.arange(P)
    perm = np.zeros((P, P), dtype=np.float32)
    perm[idx ^ 1, idx] = 1.0                 # psum[p, t] = raw[p^1, t]

    xTs = [np.ascontiguousarray(x[b].T) for b in range(B)]
    in_maps = []
    for c in range(N_CORES):
        b, g = divmod(c, N_GROUPS)
        rows = slice(FPC * g, FPC * (g + 1))
        in_maps.append({
            "xT": xTs[b],
            "wqT": np.ascontiguousarray(wq[rows].T),
            "wkT": np.ascontiguousarray(wk[rows].T),
            "wvT": np.ascontiguousarray(wv[rows].T),
            "woT": np.ascontiguousarray(wo[:, rows].T),
            "cb": cb,
            "ss": ss,
            "perm": perm,
            "ones": np.ones((P, P), dtype=np.float32),
        })
    return in_maps


def _gather(results):
    y = np.empty((B, S, DIM), dtype=np.float32)
    sub = CHUNK_TOK // N_GROUPS  # 64
    for c in range(N_CORES):
        b, r = divmod(c, N_GROUPS)
        o = results[c]["out"]  # [512, 4096]
        for ci in range(RS_CHUNKS):
            t0 = CHUNK_TOK * ci + sub * r
            y[b, t0:t0 + sub, :] = o[sub * ci: sub * (ci + 1), :]
    return y


def kernel(x, start_pos, freqs_cos, freqs_sin, wq, wk, wv, wo, trace=False):
    if "nc" not in _CACHE:
        _CACHE["nc"] = _build()
    nc = _CACHE["nc"]
    in_maps = _prep_inputs(x, freqs_cos, freqs_sin, wq, wk, wv, wo)
    res = run_bass_kernel_spmd(
        nc, in_maps, core_ids=list(range(N_CORES)), trace=trace
    )
    _CACHE["last_result"] = res
    return _gather(res.results)


# revision 16
# speedup vs baseline: 1.2428x; 1.2428x over previous
"""Trainium2 Bass kernel for nn_Attention (llama-style attention layer, fp32).

Full inputs in, full output out. 8-way tensor-parallel over heads (4 heads
per core, both batches on every core):
  - merged q/k/v projections in one pass over x (f32r matmuls, fp32 PSUM)
  - RoPE fused into projection eviction (pair-swap via permutation matmul)
  - per-head attention in [feat, tok] layout, softmax denominator via
    all-ones matmul, normalization on eviction
  - per-head AllToAll (8 cores) redistributes attention output from
    head-sharding to token-sharding, overlapped with later heads
  - output projection streams the full wo, writing [dout, tok] directly
"""
import os
import sys

sys.path.insert(0, "/opt/trn_rl_repo")

import numpy as np

import concourse.bass as bass
import concourse.mybir as mybir
import concourse.tile as tile
from concourse import bacc
from concourse.bass import ds, ts
from concourse.bass_utils import run_bass_kernel_spmd

DIM = 4096
N_HEADS = 32
HEAD_DIM = 128
B, S = 2, 2048
TOK = B * S                   # 4096 global tokens
N_CORES = 8
HPC = N_HEADS // N_CORES      # heads per core = 4
FPC = HPC * HEAD_DIM          # features per core = 512
P = 128
KO = DIM // P                 # 32 k-tiles over the model dim
NSTRIPE = TOK // 1024         # 4 projection stripes of 1024 tokens
SCALE = 1.0 / float(np.sqrt(HEAD_DIM))

f32 = mybir.dt.float32
f32r = mybir.dt.float32r
EXP = mybir.ActivationFunctionType.Exp
COPY = mybir.ActivationFunctionType.Copy
MULT = mybir.AluOpType.mult
ADD = mybir.AluOpType.add

_CACHE = {}


def _build():
    nc = bacc.Bacc(
        "TRN2", target_bir_lowering=False, debug=False, num_devices=N_CORES
    )

    xT = nc.dram_tensor("xT", [DIM, TOK], f32r, kind="ExternalInput")
    wqT = nc.dram_tensor("wqT", [DIM, FPC], f32r, kind="ExternalInput")
    wkT = nc.dram_tensor("wkT", [DIM, FPC], f32r, kind="ExternalInput")
    wvT = nc.dram_tensor("wvT", [DIM, FPC], f32r, kind="ExternalInput")
    woT = nc.dram_tensor("woT", [DIM, DIM], f32r, kind="ExternalInput")
    cb_d = nc.dram_tensor("cb", [P, S], f32, kind="ExternalInput")
    ss_d = nc.dram_tensor("ss", [P, S], f32, kind="ExternalInput")
    perm_d = nc.dram_tensor("perm", [P, P], f32r, kind="ExternalInput")
    ones_d = nc.dram_tensor("ones", [P, P], f32r, kind="ExternalInput")
    id_d = nc.dram_tensor("ident", [P, P], f32r, kind="ExternalInput")
    out_e = nc.dram_tensor("out", [DIM, TOK // N_CORES], f32, kind="ExternalOutput")

    xT3 = xT.ap().rearrange("(ko p) t -> p ko t", p=P)       # [128, 32, 4096]
    wq3 = wqT.ap().rearrange("(ko p) m -> p ko m", p=P)      # [128, 32, 512]
    wk3 = wkT.ap().rearrange("(ko p) m -> p ko m", p=P)
    wv3 = wvT.ap().rearrange("(ko p) m -> p ko m", p=P)
    wo4 = woT.ap().rearrange("(g j p) d -> p j g d", j=HPC, p=P)  # [128,4,8,4096]
    oe3 = out_e.ap().rearrange("(no p) t -> p no t", p=P)    # [128, 32, 512]

    with tile.TileContext(nc) as tc:
        with tc.tile_pool(name="dram", bufs=1, space="DRAM") as drp, \
             tc.tile_pool(name="const", bufs=1) as constp:
            q_d = drp.tile([FPC, TOK], f32r, tag="q_d", name="q_d")
            k_d = drp.tile([FPC, TOK], f32r, tag="k_d", name="k_d")
            v_d = drp.tile([FPC, TOK], f32r, tag="v_d", name="v_d")
            cc_in = [
                drp.tile([N_CORES * P, 512], f32r, tag=f"cci{j}", name=f"cci{j}")
                for j in range(HPC)
            ]
            cc_out = [
                drp.tile([N_CORES * P, 512], f32r, tag=f"cco{j}", name=f"cco{j}")
                for j in range(HPC)
            ]

            q3 = q_d[:].rearrange("(mo p) t -> p mo t", p=P)     # [128, 4, 4096]
            k3 = k_d[:].rearrange("(mo p) t -> p mo t", p=P)
            v3 = v_d[:].rearrange("(mo p) t -> p mo t", p=P)
            cci3 = [c[:].rearrange("(r p) t -> p r t", p=P) for c in cc_in]
            cco3 = [c[:].rearrange("(g p) t -> p g t", p=P) for c in cc_out]

            perm_sb = constp.tile([P, P], f32r, tag="perm", name="perm_sb")
            nc.sync.dma_start(perm_sb[:], perm_d.ap())
            ones_sb = constp.tile([P, P], f32r, tag="ones", name="ones_sb")
            nc.sync.dma_start(ones_sb[:], ones_d.ap())
            id_sb = constp.tile([P, P], f32r, tag="ident", name="id_sb")
            nc.sync.dma_start(id_sb[:], id_d.ap())

            # ---------- Phase 1: merged Q/K/V projections (+RoPE on q,k) ----
            with tc.tile_pool(name="p1_rope", bufs=1) as ropep, \
                 tc.tile_pool(name="p1_x", bufs=36) as xp, \
                 tc.tile_pool(name="p1_w", bufs=8) as wp, \
                 tc.tile_pool(name="p1_t", bufs=2) as tp, \
                 tc.tile_pool(name="p1_ps", bufs=4, space="PSUM") as pp, \
                 tc.tile_pool(name="p1_ps2", bufs=2, space="PSUM") as pp2:
                cb_sb = ropep.tile([P, S], f32, tag="cb", name="cb_sb")
                ss_sb = ropep.tile([P, S], f32, tag="ss", name="ss_sb")
                nc.sync.dma_start(cb_sb[:], cb_d.ap())
                nc.sync.dma_start(ss_sb[:], ss_d.ap())

                for n in range(NSTRIPE):  # 4 stripes of 1024 tokens
                    xs = [
                        xp.tile([P, 1024], f32r, tag="xsl", name="xs")
                        for _ in range(KO)
                    ]
                    for k in range(KO):
                        nc.sync.dma_start(xs[k][:], xT3[:, k, ts(n, 1024)])
                    for w3, dst3, rope in (
                        (wq3, q3, True), (wk3, k3, True), (wv3, v3, False)
                    ):
                        for m in range(HPC):  # 4 feature tiles (heads)
                            ps_a = pp.tile([P, 512], f32, tag="proj", name="ps_a")
                            ps_b = pp.tile([P, 512], f32, tag="proj", name="ps_b")
                            for kg in range(8):
                                wt = wp.tile([P, 4, P], f32r, tag="wt", name="wt")
                                nc.sync.dma_start(
                                    wt[:], w3[:, ts(kg, 4), ts(m, P)]
                                )
                                for kk in range(4):
                                    k = 4 * kg + kk
                                    nc.tensor.matmul(
                                        ps_a[:], wt[:, kk], xs[k][:, 0:512],
                                        start=(k == 0), stop=(k == KO - 1),
                                    )
                                    nc.tensor.matmul(
                                        ps_b[:], wt[:, kk], xs[k][:, 512:1024],
                                        start=(k == 0), stop=(k == KO - 1),
                                    )
                            for ci, pscur in ((0, ps_a), (1, ps_b)):
                                tok0 = 1024 * n + 512 * ci
                                rtok = tok0 % S  # rope tables repeat per batch
                                if rope:
                                    raw = tp.tile([P, 512], f32r, tag="raw",
                                                  name="raw")
                                    nc.scalar.activation(raw[:], pscur[:], COPY)
                                    ps_sw = pp2.tile([P, 512], f32, tag="permps",
                                                     name="ps_sw")
                                    nc.tensor.matmul(
                                        ps_sw[:], perm_sb[:], raw[:],
                                        start=True, stop=True,
                                    )
                                    qf = tp.tile([P, 512], f32r, tag="qf",
                                                 name="qf")
                                    nc.vector.tensor_tensor(
                                        qf[:], raw[:], cb_sb[:, ds(rtok, 512)],
                                        MULT,
                                    )
                                    tmp = tp.tile([P, 512], f32, tag="tmp",
                                                  name="tmp")
                                    nc.vector.tensor_tensor(
                                        tmp[:], ps_sw[:], ss_sb[:, ds(rtok, 512)],
                                        MULT,
                                    )
                                    nc.vector.tensor_tensor(
                                        qf[:], qf[:], tmp[:], ADD
                                    )
                                    nc.sync.dma_start(
                                        dst3[:, m, ds(tok0, 512)], qf[:]
                                    )
                                else:
                                    vs = tp.tile([P, 512], f32r, tag="vsb",
                                                 name="vs")
                                    nc.scalar.activation(vs[:], pscur[:], COPY)
                                    nc.sync.dma_start(
                                        dst3[:, m, ds(tok0, 512)], vs[:]
                                    )

            # ---------- Phase 3: attention per (head, batch) + AllToAll -----
            with tc.tile_pool(name="bridge", bufs=1) as brp:
              # at2[j]: post-AllToAll attention features, resident into phase 4
              at2 = [
                  brp.tile([P, N_CORES, 512], f32r, tag=f"at2_{j}",
                           name=f"at2_{j}")
                  for j in range(HPC)
              ]
              with tc.tile_pool(name="p3_kqv", bufs=2) as hp, \
                 tc.tile_pool(name="p3_exp", bufs=20) as ep, \
                 tc.tile_pool(name="p3_o", bufs=4) as aop, \
                 tc.tile_pool(name="p3_ps_s", bufs=2, space="PSUM") as sps, \
                 tc.tile_pool(name="p3_ps_o", bufs=2, space="PSUM") as ops, \
                 tc.tile_pool(name="p3_ps_d", bufs=1, space="PSUM") as dps, \
                 tc.tile_pool(name="p3_ps_t", bufs=2, space="PSUM") as tps:
                for h in range(HPC):
                    for b in range(B):
                        kh = hp.tile([P, S], f32r, tag="kh", name="kh")
                        nc.sync.dma_start(kh[:], k3[:, h, ts(b, S)])
                        qh = hp.tile([P, S], f32r, tag="qh", name="qh")
                        nc.sync.dma_start(qh[:], q3[:, h, ts(b, S)])
                        vraw = hp.tile([P, S], f32r, tag="vraw", name="vraw")
                        nc.sync.dma_start(vraw[:], v3[:, h, ts(b, S)])
                        vh = hp.tile([P, S // P, P], f32r, tag="vh", name="vh")
                        for kt in range(S // P):
                            ps_t = tps.tile([P, P], f32r, tag="pst", name="ps_t")
                            nc.tensor.transpose(
                                ps_t[:], vraw[:, ts(kt, P)], id_sb[:]
                            )
                            nc.scalar.activation(vh[:, kt], ps_t[:], COPY)
                        for qt in range(4):  # 512-token chunks within batch
                            ets = []
                            for kt in range(S // P):
                                ps_s = sps.tile([P, 512], f32, tag="s",
                                                name="ps_s")
                                nc.tensor.matmul(
                                    ps_s[:], kh[:, ts(kt, P)], qh[:, ts(qt, 512)],
                                    start=True, stop=True,
                                )
                                et = ep.tile([P, 512], f32r, tag="e", name="et")
                                nc.scalar.activation(
                                    et[:], ps_s[:], EXP, scale=SCALE
                                )
                                ets.append(et)
                            ps_o = ops.tile([P, 512], f32, tag="o", name="ps_o")
                            for kt in range(S // P):
                                nc.tensor.matmul(
                                    ps_o[:], vh[:, kt], ets[kt][:],
                                    start=(kt == 0), stop=(kt == S // P - 1),
                                )
                            ps_d = dps.tile([P, 512], f32, tag="d", name="ps_d")
                            for kt in range(S // P):
                                nc.tensor.matmul(
                                    ps_d[:], ones_sb[:], ets[kt][:],
                                    start=(kt == 0), stop=(kt == S // P - 1),
                                )
                            rec = aop.tile([P, 512], f32, tag="rec", name="rec")
                            nc.vector.reciprocal_approx_fast(rec[:], ps_d[:])
                            ao = aop.tile([P, 512], f32r, tag="ao", name="ao")
                            nc.vector.tensor_tensor(ao[:], ps_o[:], rec[:], MULT)
                            nc.sync.dma_start(
                                cci3[h][:, 4 * b + qt, :], ao[:]
                            )
                    # all 8 token-chunks of head h written -> redistribute
                    nc.gpsimd.collective_compute(
                        "AllToAll",
                        mybir.AluOpType.bypass,
                        replica_groups=[list(range(N_CORES))],
                        ins=[cc_in[h][:]],
                        outs=[cc_out[h][:]],
                    )
                    nc.sync.dma_start(at2[h][:], cco3[h])

              # ---------- Phase 4: output projection (full wo, streamed) ----
              with tc.tile_pool(name="p4_w", bufs=8) as wop, \
                   tc.tile_pool(name="p4_s", bufs=6) as osp, \
                   tc.tile_pool(name="p4_ps", bufs=3, space="PSUM") as opp:
                for nt in range(DIM // P):  # 32 dout tiles
                    psum = opp.tile([P, 512], f32, tag="ops", name="psum")
                    for j in range(HPC):
                        w8 = wop.tile([P, N_CORES, P], f32r, tag="w8", name="w8")
                        nc.sync.dma_start(w8[:], wo4[:, j, :, ts(nt, P)])
                        for g in range(N_CORES):
                            nc.tensor.matmul(
                                psum[:], w8[:, g], at2[j][:, g],
                                start=(j == 0 and g == 0),
                                stop=(j == HPC - 1 and g == N_CORES - 1),
                            )
                    ob = osp.tile([P, 512], f32, tag="ob", name="ob")
                    nc.scalar.activation(ob[:], psum[:], COPY)
                    nc.sync.dma_start(oe3[:, nt], ob[:])

    nc.compile()
    return nc


def _prep_inputs(x, freqs_cos, freqs_sin, wq, wk, wv, wo):
    x = np.asarray(x, dtype=np.float32)
    fc = np.asarray(freqs_cos, dtype=np.float32)
    fs = np.asarray(freqs_sin, dtype=np.float32)
    wq = np.asarray(wq, dtype=np.float32)
    wk = np.asarray(wk, dtype=np.float32)
    wv = np.asarray(wv, dtype=np.float32)
    wo = np.asarray(wo, dtype=np.float32)

    cb = np.ascontiguousarray(np.repeat(fc.T, 2, axis=0))  # [128,S]: cos[t,p//2]
    ss = np.repeat(fs.T, 2, axis=0)                        # [128, S]
    ss[0::2, :] *= -1.0                      # even rows: -sin, odd rows: +sin
    ss = np.ascontiguousarray(ss, dtype=np.float32)

    idx = np.arange(P)
    perm = np.zeros((P, P), dtype=np.float32)
    perm[idx ^ 1, idx] = 1.0                 # psum[p, t] = raw[p^1, t]
    ones = np.ones((P, P), dtype=np.float32)
    ident = np.eye(P, dtype=np.float32)

    xTf = np.ascontiguousarray(x.reshape(TOK, DIM).T)
    woTf = np.ascontiguousarray(wo.T)
    in_maps = []
    for c in range(N_CORES):
        rows = slice(FPC * c, FPC * (c + 1))
        in_maps.append({
            "xT": xTf,
            "wqT": np.ascontiguousarray(wq[rows].T),
            "wkT": np.ascontiguousarray(wk[rows].T),
            "wvT": np.ascontiguousarray(wv[rows].T),
            "woT": woTf,
            "cb": cb,
            "ss": ss,
            "perm": perm,
            "ones": ones,
            "ident": ident,
        })
    return in_maps


def _gather(results):
    y = np.empty((B, S, DIM), dtype=np.float32)
    for c in range(N_CORES):
        b, r = divmod(c, N_CORES // B)
        o = results[c]["out"]  # [4096 dout, 512 tok]
        y[b, 512 * r:512 * (r + 1), :] = o.T
    return y


def kernel(x, start_pos, freqs_cos, freqs_sin, wq, wk, wv, wo, trace=False):
    if "nc" not in _CACHE:
        _CACHE["nc"] = _build()
    nc = _CACHE["nc"]
    in_maps = _prep_inputs(x, freqs_cos, freqs_sin, wq, wk, wv, wo)
    res = run_bass_kernel_spmd(
        nc, in_maps, core_ids=list(range(N_CORES)), trace=trace
    )
    _CACHE["last_result"] = res
    return _gather(res.results)


# revision 22
# speedup vs baseline: 1.2444x; 1.0012x over previous
"""Trainium2 Bass kernel for nn_Attention (llama-style attention layer, fp32).

Full inputs in, full output out. 8-way tensor-parallel over heads (4 heads
per core, both batches on every core):
  - merged q/k/v projections in one pass over x (f32r matmuls, fp32 PSUM)
  - RoPE fused into projection eviction (pair-swap via permutation matmul)
  - per-head attention in [feat, tok] layout, softmax denominator via
    all-ones matmul, normalization on eviction
  - per-head AllToAll (8 cores) redistributes attention output from
    head-sharding to token-sharding, overlapped with later heads
  - output projection streams the full wo, writing [dout, tok] directly
"""
import os
import sys

sys.path.insert(0, "/opt/trn_rl_repo")

import numpy as np

import concourse.bass as bass
import concourse.mybir as mybir
import concourse.tile as tile
from concourse import bacc
from concourse.bass import ds, ts
from concourse.bass_utils import run_bass_kernel_spmd

DIM = 4096
N_HEADS = 32
HEAD_DIM = 128
B, S = 2, 2048
TOK = B * S                   # 4096 global tokens
N_CORES = 8
HPC = N_HEADS // N_CORES      # heads per core = 4
FPC = HPC * HEAD_DIM          # features per core = 512
P = 128
KO = DIM // P                 # 32 k-tiles over the model dim
NSTRIPE = TOK // 1024         # 4 projection stripes of 1024 tokens
SCALE = 1.0 / float(np.sqrt(HEAD_DIM))

f32 = mybir.dt.float32
f32r = mybir.dt.float32r
EXP = mybir.ActivationFunctionType.Exp
COPY = mybir.ActivationFunctionType.Copy
MULT = mybir.AluOpType.mult
ADD = mybir.AluOpType.add

_CACHE = {}


def _build():
    nc = bacc.Bacc(
        "TRN2", target_bir_lowering=False, debug=False, num_devices=N_CORES
    )

    xT = nc.dram_tensor("xT", [DIM, TOK], f32r, kind="ExternalInput")
    wqT = nc.dram_tensor("wqT", [DIM, FPC], f32r, kind="ExternalInput")
    wkT = nc.dram_tensor("wkT", [DIM, FPC], f32r, kind="ExternalInput")
    wvT = nc.dram_tensor("wvT", [DIM, FPC], f32r, kind="ExternalInput")
    woT = nc.dram_tensor("woT", [DIM, DIM], f32r, kind="ExternalInput")
    cb_d = nc.dram_tensor("cb", [P, S], f32, kind="ExternalInput")
    ss_d = nc.dram_tensor("ss", [P, S], f32, kind="ExternalInput")
    perm_d = nc.dram_tensor("perm", [P, P], f32r, kind="ExternalInput")
    ones_d = nc.dram_tensor("ones", [P, P], f32r, kind="ExternalInput")
    id_d = nc.dram_tensor("ident", [P, P], f32r, kind="ExternalInput")
    out_e = nc.dram_tensor("out", [DIM, TOK // N_CORES], f32, kind="ExternalOutput")

    xT3 = xT.ap().rearrange("(ko p) t -> p ko t", p=P)       # [128, 32, 4096]
    wq3 = wqT.ap().rearrange("(ko p) m -> p ko m", p=P)      # [128, 32, 512]
    wk3 = wkT.ap().rearrange("(ko p) m -> p ko m", p=P)
    wv3 = wvT.ap().rearrange("(ko p) m -> p ko m", p=P)
    wo4 = woT.ap().rearrange("(g j p) d -> p j g d", j=HPC, p=P)  # [128,4,8,4096]
    oe3 = out_e.ap().rearrange("(no p) t -> p no t", p=P)    # [128, 32, 512]

    with tile.TileContext(nc) as tc:
        with tc.tile_pool(name="dram", bufs=1, space="DRAM") as drp, \
             tc.tile_pool(name="const", bufs=1) as constp:
            q_d = drp.tile([FPC, TOK], f32r, tag="q_d", name="q_d")
            k_d = drp.tile([FPC, TOK], f32r, tag="k_d", name="k_d")
            v_d = drp.tile([FPC, TOK], f32r, tag="v_d", name="v_d")
            cc_in = [
                drp.tile([N_CORES * P, 512], f32r, tag=f"cci{j}", name=f"cci{j}")
                for j in range(HPC)
            ]
            cc_out = [
                drp.tile([N_CORES * P, 512], f32r, tag=f"cco{j}", name=f"cco{j}")
                for j in range(HPC)
            ]

            q3 = q_d[:].rearrange("(mo p) t -> p mo t", p=P)     # [128, 4, 4096]
            k3 = k_d[:].rearrange("(mo p) t -> p mo t", p=P)
            v3 = v_d[:].rearrange("(mo p) t -> p mo t", p=P)
            cci3 = [c[:].rearrange("(r p) t -> p r t", p=P) for c in cc_in]
            cco3 = [c[:].rearrange("(g p) t -> p g t", p=P) for c in cc_out]

            perm_sb = constp.tile([P, P], f32r, tag="perm", name="perm_sb")
            nc.sync.dma_start(perm_sb[:], perm_d.ap())
            ones_sb = constp.tile([P, P], f32r, tag="ones", name="ones_sb")
            nc.sync.dma_start(ones_sb[:], ones_d.ap())
            id_sb = constp.tile([P, P], f32r, tag="ident", name="id_sb")
            nc.sync.dma_start(id_sb[:], id_d.ap())

            # ---------- Phase 1: merged Q/K/V projections (+RoPE on q,k) ----
            with tc.tile_pool(name="p1_rope", bufs=1) as ropep, \
                 tc.tile_pool(name="p1_x", bufs=40) as xp, \
                 tc.tile_pool(name="p1_w", bufs=6) as wp, \
                 tc.tile_pool(name="p1_t", bufs=2) as tp, \
                 tc.tile_pool(name="p1_ps", bufs=4, space="PSUM") as pp, \
                 tc.tile_pool(name="p1_ps2", bufs=2, space="PSUM") as pp2:
                cb_sb = ropep.tile([P, S], f32, tag="cb", name="cb_sb")
                ss_sb = ropep.tile([P, S], f32, tag="ss", name="ss_sb")
                nc.sync.dma_start(cb_sb[:], cb_d.ap())
                nc.sync.dma_start(ss_sb[:], ss_d.ap())

                for n in range(NSTRIPE):  # 4 stripes of 1024 tokens
                    xs = [
                        xp.tile([P, 1024], f32r, tag="xsl", name="xs")
                        for _ in range(KO)
                    ]
                    for k in range(KO):
                        nc.sync.dma_start(xs[k][:], xT3[:, k, ts(n, 1024)])
                    for w3, dst3, rope in (
                        (wq3, q3, True), (wk3, k3, True), (wv3, v3, False)
                    ):
                        for m in range(HPC):  # 4 feature tiles (heads)
                            ps_a = pp.tile([P, 512], f32, tag="proj", name="ps_a")
                            ps_b = pp.tile([P, 512], f32, tag="proj", name="ps_b")
                            for kg in range(8):
                                wt = wp.tile([P, 4, P], f32r, tag="wt", name="wt")
                                nc.sync.dma_start(
                                    wt[:], w3[:, ts(kg, 4), ts(m, P)]
                                )
                                for kk in range(4):
                                    k = 4 * kg + kk
                                    nc.tensor.matmul(
                                        ps_a[:], wt[:, kk], xs[k][:, 0:512],
                                        start=(k == 0), stop=(k == KO - 1),
                                    )
                                    nc.tensor.matmul(
                                        ps_b[:], wt[:, kk], xs[k][:, 512:1024],
                                        start=(k == 0), stop=(k == KO - 1),
                                    )
                            for ci, pscur in ((0, ps_a), (1, ps_b)):
                                tok0 = 1024 * n + 512 * ci
                                rtok = tok0 % S  # rope tables repeat per batch
                                if rope:
                                    raw = tp.tile([P, 512], f32r, tag="raw",
                                                  name="raw")
                                    nc.scalar.activation(raw[:], pscur[:], COPY)
                                    ps_sw = pp2.tile([P, 512], f32, tag="permps",
                                                     name="ps_sw")
                                    nc.tensor.matmul(
                                        ps_sw[:], perm_sb[:], raw[:],
                                        start=True, stop=True,
                                    )
                                    qf = tp.tile([P, 512], f32r, tag="qf",
                                                 name="qf")
                                    nc.vector.tensor_tensor(
                                        qf[:], raw[:], cb_sb[:, ds(rtok, 512)],
                                        MULT,
                                    )
                                    tmp = tp.tile([P, 512], f32, tag="tmp",
                                                  name="tmp")
                                    nc.vector.tensor_tensor(
                                        tmp[:], ps_sw[:], ss_sb[:, ds(rtok, 512)],
                                        MULT,
                                    )
                                    nc.vector.tensor_tensor(
                                        qf[:], qf[:], tmp[:], ADD
                                    )
                                    nc.sync.dma_start(
                                        dst3[:, m, ds(tok0, 512)], qf[:]
                                    )
                                else:
                                    vs = tp.tile([P, 512], f32r, tag="vsb",
                                                 name="vs")
                                    nc.scalar.activation(vs[:], pscur[:], COPY)
                                    nc.sync.dma_start(
                                        dst3[:, m, ds(tok0, 512)], vs[:]
                                    )

            # ---------- Phase 3: attention per (head, batch) + AllToAll -----
            with tc.tile_pool(name="bridge", bufs=1) as brp:
              # at2[j]: post-AllToAll attention features, resident into phase 4
              at2 = [
                  brp.tile([P, N_CORES, 512], f32r, tag=f"at2_{j}",
                           name=f"at2_{j}")
                  for j in range(HPC)
              ]
              with tc.tile_pool(name="p3_kqv", bufs=2) as hp, \
                 tc.tile_pool(name="p3_exp", bufs=10) as ep, \
                 tc.tile_pool(name="p3_o", bufs=4) as aop, \
                 tc.tile_pool(name="p3_ps_s", bufs=2, space="PSUM") as sps, \
                 tc.tile_pool(name="p3_ps_o", bufs=1, space="PSUM") as ops, \
                 tc.tile_pool(name="p3_ps_d", bufs=1, space="PSUM") as dps, \
                 tc.tile_pool(name="p3_ps_t", bufs=2, space="PSUM") as tps:
                for h in range(HPC):
                    for b in range(B):
                        kh = hp.tile([P, S], f32r, tag="kh", name="kh")
                        nc.sync.dma_start(kh[:], k3[:, h, ts(b, S)])
                        qh = hp.tile([P, S], f32r, tag="qh", name="qh")
                        nc.sync.dma_start(qh[:], q3[:, h, ts(b, S)])
                        vraw = hp.tile([P, S], f32r, tag="vraw", name="vraw")
                        nc.sync.dma_start(vraw[:], v3[:, h, ts(b, S)])
                        vh = hp.tile([P, S // P, P], f32r, tag="vh", name="vh")
                        for kt in range(S // P):
                            ps_t = tps.tile([P, P], f32r, tag="pst", name="ps_t")
                            nc.tensor.transpose(
                                ps_t[:], vraw[:, ts(kt, P)], id_sb[:]
                            )
                            nc.vector.tensor_copy(out=vh[:, kt], in_=ps_t[:])
                        for qt in range(4):  # 512-token chunks within batch
                            ets = []
                            for k2 in range(S // P // 2):  # pairs of ktok tiles
                                ps_s = sps.tile([P, 1024], f32, tag="s",
                                                name="ps_s")
                                for kk in range(2):
                                    kt = 2 * k2 + kk
                                    nc.tensor.matmul(
                                        ps_s[:, ts(kk, 512)],
                                        kh[:, ts(kt, P)], qh[:, ts(qt, 512)],
                                        start=True, stop=True,
                                    )
                                et = ep.tile([P, 1024], f32r, tag="e", name="et")
                                nc.scalar.activation(
                                    et[:], ps_s[:], EXP, scale=SCALE
                                )
                                ets.append(et)
                            ps_o = ops.tile([P, 512], f32, tag="o", name="ps_o")
                            for kt in range(S // P):
                                nc.tensor.matmul(
                                    ps_o[:], vh[:, kt],
                                    ets[kt // 2][:, ts(kt % 2, 512)],
                                    start=(kt == 0), stop=(kt == S // P - 1),
                                )
                            ps_d = dps.tile([P, 512], f32, tag="d", name="ps_d")
                            for kt in range(S // P):
                                nc.tensor.matmul(
                                    ps_d[:], ones_sb[:],
                                    ets[kt // 2][:, ts(kt % 2, 512)],
                                    start=(kt == 0), stop=(kt == S // P - 1),
                                )
                            rec = aop.tile([P, 512], f32, tag="rec", name="rec")
                            nc.vector.reciprocal_approx_fast(rec[:], ps_d[:])
                            ao = aop.tile([P, 512], f32r, tag="ao", name="ao")
                            nc.vector.tensor_tensor(ao[:], ps_o[:], rec[:], MULT)
                            nc.sync.dma_start(
                                cci3[h][:, 4 * b + qt, :], ao[:]
                            )
                    # all 8 token-chunks of head h written -> redistribute
                    nc.gpsimd.collective_compute(
                        "AllToAll",
                        mybir.AluOpType.bypass,
                        replica_groups=[list(range(N_CORES))],
                        ins=[cc_in[h][:]],
                        outs=[cc_out[h][:]],
                    )
                    nc.sync.dma_start(at2[h][:], cco3[h])

              # ---------- Phase 4: output projection (full wo, streamed) ----
              with tc.tile_pool(name="p4_w", bufs=8) as wop, \
                   tc.tile_pool(name="p4_s", bufs=6) as osp, \
                   tc.tile_pool(name="p4_ps", bufs=6, space="PSUM") as opp:
                for nt in range(DIM // P):  # 32 dout tiles
                    psum = opp.tile([P, 512], f32, tag="ops", name="psum")
                    for j in range(HPC):
                        w8 = wop.tile([P, N_CORES, P], f32r, tag="w8", name="w8")
                        nc.sync.dma_start(w8[:], wo4[:, j, :, ts(nt, P)])
                        for g in range(N_CORES):
                            nc.tensor.matmul(
                                psum[:], w8[:, g], at2[j][:, g],
                                start=(j == 0 and g == 0),
                                stop=(j == HPC - 1 and g == N_CORES - 1),
                            )
                    ob = osp.tile([P, 512], f32, tag="ob", name="ob")
                    nc.scalar.activation(ob[:], psum[:], COPY)
                    nc.sync.dma_start(oe3[:, nt], ob[:])

    nc.compile()
    return nc


def _prep_inputs(x, freqs_cos, freqs_sin, wq, wk, wv, wo):
    x = np.asarray(x, dtype=np.float32)
    fc = np.asarray(freqs_cos, dtype=np.float32)
    fs = np.asarray(freqs_sin, dtype=np.float32)
    wq = np.asarray(wq, dtype=np.float32)
    wk = np.asarray(wk, dtype=np.float32)
    wv = np.asarray(wv, dtype=np.float32)
    wo = np.asarray(wo, dtype=np.float32)

    cb = np.ascontiguousarray(np.repeat(fc.T, 2, axis=0))  # [128,S]: cos[t,p//2]
    ss = np.repeat(fs.T, 2, axis=0)                        # [128, S]
    ss[0::2, :] *= -1.0                      # even rows: -sin, odd rows: +sin
    ss = np.ascontiguousarray(ss, dtype=np.float32)

    idx = np.arange(P)
    perm = np.zeros((P, P), dtype=np.float32)
    perm[idx ^ 1, idx] = 1.0                 # psum[p, t] = raw[p^1, t]
    ones = np.ones((P, P), dtype=np.float32)
    ident = np.eye(P, dtype=np.float32)

    xTf = np.ascontiguousarray(x.reshape(TOK, DIM).T)
    woTf = np.ascontiguousarray(wo.T)
    in_maps = []
    for c in range(N_CORES):
        rows = slice(FPC * c, FPC * (c + 1))
        in_maps.append({
            "xT": xTf,
            "wqT": np.ascontiguousarray(wq[rows].T),
            "wkT": np.ascontiguousarray(wk[rows].T),
            "wvT": np.ascontiguousarray(wv[rows].T),
            "woT": woTf,
            "cb": cb,
            "ss": ss,
            "perm": perm,
            "ones": ones,
            "ident": ident,
        })
    return in_maps


def _gather(results):
    y = np.empty((B, S, DIM), dtype=np.float32)
    for c in range(N_CORES):
        b, r = divmod(c, N_CORES // B)
        o = results[c]["out"]  # [4096 dout, 512 tok]
        y[b, 512 * r:512 * (r + 1), :] = o.T
    return y


def kernel(x, start_pos, freqs_cos, freqs_sin, wq, wk, wv, wo, trace=False):
    if "nc" not in _CACHE:
        _CACHE["nc"] = _build()
    nc = _CACHE["nc"]
    in_maps = _prep_inputs(x, freqs_cos, freqs_sin, wq, wk, wv, wo)
    res = run_bass_kernel_spmd(
        nc, in_maps, core_ids=list(range(N_CORES)), trace=trace
    )
    _CACHE["last_result"] = res
    return _gather(res.results)


# revision 25
# speedup vs baseline: 1.2675x; 1.0186x over previous
"""Trainium2 Bass kernel for nn_Attention (llama-style attention layer, fp32).

Full inputs in, full output out. 8-way tensor-parallel over heads (4 heads
per core, both batches on every core):
  - merged q/k/v projections in one pass over x (f32r matmuls, fp32 PSUM)
  - RoPE fused into projection eviction (pair-swap via permutation matmul)
  - per-head attention in [feat, tok] layout, softmax denominator via
    all-ones matmul, normalization on eviction
  - per-head AllToAll (8 cores) redistributes attention output from
    head-sharding to token-sharding, overlapped with later heads
  - output projection streams the full wo, writing [dout, tok] directly
"""
import os
import sys

sys.path.insert(0, "/opt/trn_rl_repo")

import numpy as np

import concourse.bass as bass
import concourse.mybir as mybir
import concourse.tile as tile
from concourse import bacc
from concourse.bass import ds, ts
from concourse.bass_utils import run_bass_kernel_spmd

DIM = 4096
N_HEADS = 32
HEAD_DIM = 128
B, S = 2, 2048
TOK = B * S                   # 4096 global tokens
N_CORES = 8
HPC = N_HEADS // N_CORES      # heads per core = 4
FPC = HPC * HEAD_DIM          # features per core = 512
P = 128
KO = DIM // P                 # 32 k-tiles over the model dim
NSTRIPE = TOK // 1024         # 4 projection stripes of 1024 tokens
SCALE = 1.0 / float(np.sqrt(HEAD_DIM))

f32 = mybir.dt.float32
f32r = mybir.dt.float32r
EXP = mybir.ActivationFunctionType.Exp
COPY = mybir.ActivationFunctionType.Copy
MULT = mybir.AluOpType.mult
ADD = mybir.AluOpType.add

_CACHE = {}


def _build():
    nc = bacc.Bacc(
        "TRN2", target_bir_lowering=False, debug=False, num_devices=N_CORES
    )

    xT = nc.dram_tensor("xT", [DIM, TOK], f32r, kind="ExternalInput")
    wqT = nc.dram_tensor("wqT", [DIM, FPC], f32r, kind="ExternalInput")
    wkT = nc.dram_tensor("wkT", [DIM, FPC], f32r, kind="ExternalInput")
    wvT = nc.dram_tensor("wvT", [DIM, FPC], f32r, kind="ExternalInput")
    woT = nc.dram_tensor("woT", [DIM, DIM], f32r, kind="ExternalInput")
    cb_d = nc.dram_tensor("cb", [P, S], f32, kind="ExternalInput")
    ss_d = nc.dram_tensor("ss", [P, S], f32, kind="ExternalInput")
    perm_d = nc.dram_tensor("perm", [P, P], f32r, kind="ExternalInput")
    ones_d = nc.dram_tensor("ones", [P, P], f32r, kind="ExternalInput")
    id_d = nc.dram_tensor("ident", [P, P], f32r, kind="ExternalInput")
    out_e = nc.dram_tensor("out", [DIM, TOK // N_CORES], f32, kind="ExternalOutput")

    xT3 = xT.ap().rearrange("(ko p) t -> p ko t", p=P)       # [128, 32, 4096]
    wq3 = wqT.ap().rearrange("(ko p) m -> p ko m", p=P)      # [128, 32, 512]
    wk3 = wkT.ap().rearrange("(ko p) m -> p ko m", p=P)
    wv3 = wvT.ap().rearrange("(ko p) m -> p ko m", p=P)
    wo4 = woT.ap().rearrange("(g j p) d -> p j g d", j=HPC, p=P)  # [128,4,8,4096]
    oe3 = out_e.ap().rearrange("(no p) t -> p no t", p=P)    # [128, 32, 512]

    with tile.TileContext(nc) as tc:
        with tc.tile_pool(name="dram", bufs=1, space="DRAM") as drp, \
             tc.tile_pool(name="const", bufs=1) as constp:
            q_d = drp.tile([FPC, TOK], f32r, tag="q_d", name="q_d")
            k_d = drp.tile([FPC, TOK], f32r, tag="k_d", name="k_d")
            v_d = drp.tile([FPC, TOK], f32r, tag="v_d", name="v_d")
            cc_in = [
                drp.tile([N_CORES * P, 512], f32r, tag=f"cci{j}", name=f"cci{j}")
                for j in range(HPC)
            ]
            cc_out = [
                drp.tile([N_CORES * P, 512], f32r, tag=f"cco{j}", name=f"cco{j}")
                for j in range(HPC)
            ]

            q3 = q_d[:].rearrange("(mo p) t -> p mo t", p=P)     # [128, 4, 4096]
            k3 = k_d[:].rearrange("(mo p) t -> p mo t", p=P)
            v3 = v_d[:].rearrange("(mo p) t -> p mo t", p=P)
            cci3 = [c[:].rearrange("(r p) t -> p r t", p=P) for c in cc_in]
            cco3 = [c[:].rearrange("(g p) t -> p g t", p=P) for c in cc_out]

            perm_sb = constp.tile([P, P], f32r, tag="perm", name="perm_sb")
            nc.sync.dma_start(perm_sb[:], perm_d.ap())
            ones_sb = constp.tile([P, P], f32r, tag="ones", name="ones_sb")
            nc.sync.dma_start(ones_sb[:], ones_d.ap())
            id_sb = constp.tile([P, P], f32r, tag="ident", name="id_sb")
            nc.sync.dma_start(id_sb[:], id_d.ap())

            # ---------- Phase 1: merged Q/K/V projections (+RoPE on q,k) ----
            # K-split: each 1024-token stripe runs k-tiles 0..15 into partial
            # SBUF tiles, then k-tiles 16..31; eviction adds the halves. The
            # halved x live-set gives the pool real prefetch slots.
            with tc.tile_pool(name="p1_rope", bufs=1) as ropep, \
                 tc.tile_pool(name="p1_x", bufs=28) as xp, \
                 tc.tile_pool(name="p1_part", bufs=24) as prt, \
                 tc.tile_pool(name="p1_w", bufs=6) as wp, \
                 tc.tile_pool(name="p1_t", bufs=2) as tp, \
                 tc.tile_pool(name="p1_ps", bufs=4, space="PSUM") as pp, \
                 tc.tile_pool(name="p1_ps2", bufs=2, space="PSUM") as pp2:
                cb_sb = ropep.tile([P, S], f32, tag="cb", name="cb_sb")
                ss_sb = ropep.tile([P, S], f32, tag="ss", name="ss_sb")
                nc.sync.dma_start(cb_sb[:], cb_d.ap())
                nc.sync.dma_start(ss_sb[:], ss_d.ap())

                projs = ((wq3, q3, True), (wk3, k3, True), (wv3, v3, False))
                for n in range(NSTRIPE):  # 4 stripes of 1024 tokens
                    parts = {}
                    for h2 in range(2):   # contraction halves (k 0-15, 16-31)
                        xs = [
                            xp.tile([P, 1024], f32r, tag="xsl", name="xs")
                            for _ in range(KO // 2)
                        ]
                        for kl in range(KO // 2):
                            nc.sync.dma_start(
                                xs[kl][:], xT3[:, 16 * h2 + kl, ts(n, 1024)]
                            )
                        for pi, (w3, dst3, rope) in enumerate(projs):
                            for m in range(HPC):  # 4 feature tiles (heads)
                                ps_a = pp.tile([P, 512], f32, tag="proj",
                                               name="ps_a")
                                ps_b = pp.tile([P, 512], f32, tag="proj",
                                               name="ps_b")
                                for kg in range(4):
                                    wt = wp.tile([P, 4, P], f32r, tag="wt",
                                                 name="wt")
                                    nc.sync.dma_start(
                                        wt[:],
                                        w3[:, ds(16 * h2 + 4 * kg, 4), ts(m, P)],
                                    )
                                    for kk in range(4):
                                        kl = 4 * kg + kk
                                        nc.tensor.matmul(
                                            ps_a[:], wt[:, kk], xs[kl][:, 0:512],
                                            start=(kl == 0), stop=(kl == 15),
                                        )
                                        nc.tensor.matmul(
                                            ps_b[:], wt[:, kk],
                                            xs[kl][:, 512:1024],
                                            start=(kl == 0), stop=(kl == 15),
                                        )
                                for ci, pscur in ((0, ps_a), (1, ps_b)):
                                    if h2 == 0:
                                        part = prt.tile([P, 512], f32,
                                                        tag="part", name="part")
                                        nc.scalar.activation(
                                            part[:], pscur[:], COPY
                                        )
                                        parts[(pi, m, ci)] = part
                                        continue
                                    part = parts[(pi, m, ci)]
                                    tok0 = 1024 * n + 512 * ci
                                    rtok = tok0 % S  # rope tables repeat/batch
                                    if rope:
                                        raw = tp.tile([P, 512], f32r, tag="raw",
                                                      name="raw")
                                        nc.vector.tensor_tensor(
                                            raw[:], pscur[:], part[:], ADD
                                        )
                                        ps_sw = pp2.tile([P, 512], f32,
                                                         tag="permps",
                                                         name="ps_sw")
                                        nc.tensor.matmul(
                                            ps_sw[:], perm_sb[:], raw[:],
                                            start=True, stop=True,
                                        )
                                        qf = tp.tile([P, 512], f32r, tag="qf",
                                                     name="qf")
                                        nc.vector.tensor_tensor(
                                            qf[:], raw[:],
                                            cb_sb[:, ds(rtok, 512)], MULT,
                                        )
                                        tmp = tp.tile([P, 512], f32, tag="tmp",
                                                      name="tmp")
                                        nc.vector.tensor_tensor(
                                            tmp[:], ps_sw[:],
                                            ss_sb[:, ds(rtok, 512)], MULT,
                                        )
                                        nc.vector.tensor_tensor(
                                            qf[:], qf[:], tmp[:], ADD
                                        )
                                        nc.sync.dma_start(
                                            dst3[:, m, ds(tok0, 512)], qf[:]
                                        )
                                    else:
                                        vs = tp.tile([P, 512], f32r, tag="vsb",
                                                     name="vs")
                                        nc.vector.tensor_tensor(
                                            vs[:], pscur[:], part[:], ADD
                                        )
                                        nc.sync.dma_start(
                                            dst3[:, m, ds(tok0, 512)], vs[:]
                                        )

            # ---------- Phase 3: attention per (head, batch) + AllToAll -----
            with tc.tile_pool(name="bridge", bufs=1) as brp:
              # at2[j]: post-AllToAll attention features, resident into phase 4
              at2 = [
                  brp.tile([P, N_CORES, 512], f32r, tag=f"at2_{j}",
                           name=f"at2_{j}")
                  for j in range(HPC)
              ]
              with tc.tile_pool(name="p3_kqv", bufs=2) as hp, \
                 tc.tile_pool(name="p3_exp", bufs=10) as ep, \
                 tc.tile_pool(name="p3_o", bufs=4) as aop, \
                 tc.tile_pool(name="p3_ps_s", bufs=2, space="PSUM") as sps, \
                 tc.tile_pool(name="p3_ps_o", bufs=1, space="PSUM") as ops, \
                 tc.tile_pool(name="p3_ps_d", bufs=1, space="PSUM") as dps, \
                 tc.tile_pool(name="p3_ps_t", bufs=2, space="PSUM") as tps:
                for h in range(HPC):
                    for b in range(B):
                        kh = hp.tile([P, S], f32r, tag="kh", name="kh")
                        nc.sync.dma_start(kh[:], k3[:, h, ts(b, S)])
                        qh = hp.tile([P, S], f32r, tag="qh", name="qh")
                        nc.sync.dma_start(qh[:], q3[:, h, ts(b, S)])
                        vraw = hp.tile([P, S], f32r, tag="vraw", name="vraw")
                        nc.sync.dma_start(vraw[:], v3[:, h, ts(b, S)])
                        vh = hp.tile([P, S // P, P], f32r, tag="vh", name="vh")
                        for kt in range(S // P):
                            ps_t = tps.tile([P, P], f32r, tag="pst", name="ps_t")
                            nc.tensor.transpose(
                                ps_t[:], vraw[:, ts(kt, P)], id_sb[:]
                            )
                            nc.vector.tensor_copy(out=vh[:, kt], in_=ps_t[:])
                        for qt in range(4):  # 512-token chunks within batch
                            ets = []
                            for k2 in range(S // P // 2):  # pairs of ktok tiles
                                ps_s = sps.tile([P, 1024], f32, tag="s",
                                                name="ps_s")
                                for kk in range(2):
                                    kt = 2 * k2 + kk
                                    nc.tensor.matmul(
                                        ps_s[:, ts(kk, 512)],
                                        kh[:, ts(kt, P)], qh[:, ts(qt, 512)],
                                        start=True, stop=True,
                                    )
                                et = ep.tile([P, 1024], f32r, tag="e", name="et")
                                nc.scalar.activation(
                                    et[:], ps_s[:], EXP, scale=SCALE
                                )
                                ets.append(et)
                            ps_o = ops.tile([P, 512], f32, tag="o", name="ps_o")
                            for kt in range(S // P):
                                nc.tensor.matmul(
                                    ps_o[:], vh[:, kt],
                                    ets[kt // 2][:, ts(kt % 2, 512)],
                                    start=(kt == 0), stop=(kt == S // P - 1),
                                )
                            ps_d = dps.tile([P, 512], f32, tag="d", name="ps_d")
                            for kt in range(S // P):
                                nc.tensor.matmul(
                                    ps_d[:], ones_sb[:],
                                    ets[kt // 2][:, ts(kt % 2, 512)],
                                    start=(kt == 0), stop=(kt == S // P - 1),
                                )
                            rec = aop.tile([P, 512], f32, tag="rec", name="rec")
                            nc.vector.reciprocal_approx_fast(rec[:], ps_d[:])
                            ao = aop.tile([P, 512], f32r, tag="ao", name="ao")
                            nc.vector.tensor_tensor(ao[:], ps_o[:], rec[:], MULT)
                            nc.sync.dma_start(
                                cci3[h][:, 4 * b + qt, :], ao[:]
                            )
                    # all 8 token-chunks of head h written -> redistribute
                    nc.gpsimd.collective_compute(
                        "AllToAll",
                        mybir.AluOpType.bypass,
                        replica_groups=[list(range(N_CORES))],
                        ins=[cc_in[h][:]],
                        outs=[cc_out[h][:]],
                    )
                    nc.sync.dma_start(at2[h][:], cco3[h])

              # ---------- Phase 4: output projection (full wo, streamed) ----
              # Two passes: j=0..2 contributions first (only needs the first
              # three AllToAlls -> overlaps the last one), then j=3 + combine.
              with tc.tile_pool(name="p4_w", bufs=4) as wop, \
                   tc.tile_pool(name="p4_w2", bufs=4) as wop2, \
                   tc.tile_pool(name="p4_part", bufs=32) as p4p, \
                   tc.tile_pool(name="p4_s", bufs=4) as osp, \
                   tc.tile_pool(name="p4_ps", bufs=6, space="PSUM") as opp:
                oparts = []
                for nt in range(DIM // P):  # 32 dout tiles: j = 0..2
                    psum = opp.tile([P, 512], f32, tag="ops", name="psum")
                    w6 = wop.tile([P, HPC - 1, N_CORES, P], f32r, tag="w6",
                                  name="w6")
                    for j in range(HPC - 1):
                        nc.sync.dma_start(w6[:, j], wo4[:, j, :, ts(nt, P)])
                    for j in range(HPC - 1):
                        for g in range(N_CORES):
                            nc.tensor.matmul(
                                psum[:], w6[:, j, g], at2[j][:, g],
                                start=(j == 0 and g == 0),
                                stop=(j == HPC - 2 and g == N_CORES - 1),
                            )
                    op = p4p.tile([P, 512], f32, tag="opart", name="op")
                    nc.scalar.activation(op[:], psum[:], COPY)
                    oparts.append(op)
                for nt in range(DIM // P):  # j = 3 + combine + store
                    psum = opp.tile([P, 512], f32, tag="ops", name="psum")
                    w2 = wop2.tile([P, N_CORES, P], f32r, tag="w2", name="w2")
                    nc.sync.dma_start(w2[:], wo4[:, HPC - 1, :, ts(nt, P)])
                    for g in range(N_CORES):
                        nc.tensor.matmul(
                            psum[:], w2[:, g], at2[HPC - 1][:, g],
                            start=(g == 0), stop=(g == N_CORES - 1),
                        )
                    ob = osp.tile([P, 512], f32, tag="ob", name="ob")
                    nc.vector.tensor_tensor(ob[:], psum[:], oparts[nt][:], ADD)
                    nc.sync.dma_start(oe3[:, nt], ob[:])

    nc.compile()
    return nc


def _prep_inputs(x, freqs_cos, freqs_sin, wq, wk, wv, wo):
    x = np.asarray(x, dtype=np.float32)
    fc = np.asarray(freqs_cos, dtype=np.float32)
    fs = np.asarray(freqs_sin, dtype=np.float32)
    wq = np.asarray(wq, dtype=np.float32)
    wk = np.asarray(wk, dtype=np.float32)
    wv = np.asarray(wv, dtype=np.float32)
    wo = np.asarray(wo, dtype=np.float32)

    cb = np.ascontiguousarray(np.repeat(fc.T, 2, axis=0))  # [128,S]: cos[t,p//2]
    ss = np.repeat(fs.T, 2, axis=0)                        # [128, S]
    ss[0::2, :] *= -1.0                      # even rows: -sin, odd rows: +sin
    ss = np.ascontiguousarray(ss, dtype=np.float32)

    idx = np.arange(P)
    perm = np.zeros((P, P), dtype=np.float32)
    perm[idx ^ 1, idx] = 1.0                 # psum[p, t] = raw[p^1, t]
    ones = np.ones((P, P), dtype=np.float32)
    ident = np.eye(P, dtype=np.float32)

    xTf = np.ascontiguousarray(x.reshape(TOK, DIM).T)
    woTf = np.ascontiguousarray(wo.T)
    in_maps = []
    for c in range(N_CORES):
        rows = slice(FPC * c, FPC * (c + 1))
        in_maps.append({
            "xT": xTf,
            "wqT": np.ascontiguousarray(wq[rows].T),
            "wkT": np.ascontiguousarray(wk[rows].T),
            "wvT": np.ascontiguousarray(wv[rows].T),
            "woT": woTf,
            "cb": cb,
            "ss": ss,
            "perm": perm,
            "ones": ones,
            "ident": ident,
        })
    return in_maps


def _gather(results):
    y = np.empty((B, S, DIM), dtype=np.float32)
    for c in range(N_CORES):
        b, r = divmod(c, N_CORES // B)
        o = results[c]["out"]  # [4096 dout, 512 tok]
        y[b, 512 * r:512 * (r + 1), :] = o.T
    return y


def kernel(x, start_pos, freqs_cos, freqs_sin, wq, wk, wv, wo, trace=False):
    if "nc" not in _CACHE:
        _CACHE["nc"] = _build()
    nc = _CACHE["nc"]
    in_maps = _prep_inputs(x, freqs_cos, freqs_sin, wq, wk, wv, wo)
    res = run_bass_kernel_spmd(
        nc, in_maps, core_ids=list(range(N_CORES)), trace=trace
    )
    _CACHE["last_result"] = res
    return _gather(res.results)
